# revision 1
# baseline (speedup 1.0000x reference)
"""Trainium2 Bass kernel for nn_ExtractPatchesPositionLayer.

Reference semantics: per image b, bilinear-translate the (522,522,1) padded
object by t = -positions[b] (tfa.translate: out(y,x) = img(y+py, x+px),
zero fill outside), then center-crop 5px -> (512,512,1).

Because the shift is constant per image, floor/frac of the offset give an
integer window start (A,B) into the (zero-margin-padded) image plus four
constant bilinear corner weights:

    out[r, j] = c00*W[r, j] + c01*W[r, j+1] + c10*W[r+1, j] + c11*W[r+1, j+1]
    W[r, c] = pp[A+r, B+c]

Layout trick: SBUF partition p holds FOUR consecutive padded-image rows
(A+4p .. A+4p+3, +1 elem) as ONE contiguous DRAM span (4*wpad+1 elements, a
single ~8.4 KB line-rate DMA descriptor per partition).  The shared
horizontal lerp h = (1-wx)*wt + wx*wt[+1] is computed once over the whole
span in RATIO form (one DVE fused madd: g = wt + rx*wt[+1], rx = wx/(1-wx);
all CONTIGUOUS free-dim APs -- DVE runs flat APs at ~2x the rate of strided
3D ones).  The vertical lerp is partition-local (m = g + ry*g[+wpad]) except
each partition's LAST row pair, whose g row 4 == next partition's g row 0:
the otherwise-idle PE recovers it with a shift-matrix matmul (zero last
column, so ps[127,:]=0 stays defined) that the DVE madd reads straight from
PSUM.  The combined scale S=(1-wx)(1-wy) is applied LAST by ACT, which runs
strided APs at full rate and therefore writes a COMPACT 512-wide output
tile: 4 consecutive y rows per partition = one contiguous 8 KB descriptor,
no write junk, no host trim.  The very last output row (needs input row
A+512, outside the spans) is patched on host -- O(B*N) work.  Ratio form is
numerically safe here: 1-wx, 1-wy in (0,1], and the big term dominates both
g and the output, keeping error at the output's ulp scale.

DMA routing (hard-won trace facts):
  * inputs: dynamic HWDGE on the SP ring (runtime reg offsets; descriptors
    spread over all 16 SDMA engines by dest SBUF partition).
  * outputs: SWDGE via gpsimd -- HWDGE sends every SBUF->HBM descriptor to
    SDMA engine 0 (1.4 ms serialized); SWDGE's CounterMachine spreads them.
    8+ KB descriptors avoid SWDGE's 8-byte stub-packet flood seen at 2 KB.
Sharding: batch 256 -> 32 images x 8 cores, embarrassingly parallel.
Measured: 1426 us (baseline banded-matmul PE kernel) -> 181 us best-state;
the 16 SDMA engines (~165 us busy each, work-conserving at ~100% mid-run)
are the binding resource (~400+ GB/s aggregate HBM traffic, past the
documented 358 GB/s per-core); DVE ~155 us.  Device power-state drift adds
up to ~12% run-to-run.
"""

from dataclasses import dataclass

import numpy as np

import concourse.bacc as bacc
import concourse.bass as bass
import concourse.mybir as mybir
import concourse.tile as tile
from concourse.bass_utils import run_bass_kernel_spmd


@dataclass(frozen=True)
class Cfg:
    bpc: int      # images per core
    n: int        # output height/width
    wpad: int     # padded input height/width (with zero margin)
    xlen: int     # flat padded-input length per core (incl. tail pad)

    @property
    def wrow(self):  # output rows per partition
        return self.n // 128

    @property
    def span(self):  # elements DMA'd per partition (WR rows + 1)
        return self.wrow * self.wpad + 1


def build_nc(cfg: Cfg) -> bass.Bass:
    BPC, N, WPAD = cfg.bpc, cfg.n, cfg.wpad
    WR = cfg.wrow
    SPAN = cfg.span
    WIDE = WR * WPAD  # full-width output row block per partition
    XLEN = cfg.xlen
    f32 = mybir.dt.float32
    i32 = mybir.dt.int32
    MUL = mybir.AluOpType.mult
    ADD = mybir.AluOpType.add

    nc = bacc.Bacc("TRN2", target_bir_lowering=False, debug=False)
    x_d = nc.declare_dram_parameter("x", [1, XLEN], f32, isOutput=False)
    offs_d = nc.declare_dram_parameter("offs", [1, BPC], i32, isOutput=False)
    wmat_d = nc.declare_dram_parameter("wmat", [BPC, 128, 4], f32, isOutput=False)
    smat_d = nc.declare_dram_parameter("smat", [128, 128], f32, isOutput=False)
    y_d = nc.declare_dram_parameter("y", [BPC, N, N], f32, isOutput=True)

    with tile.TileContext(nc) as tc:
        with (
            tc.tile_pool(name="const", bufs=1) as constp,
            tc.tile_pool(name="win", bufs=8) as winp,
            tc.tile_pool(name="hp", bufs=5) as hp,
            tc.tile_pool(name="mp", bufs=4) as mp,
            tc.tile_pool(name="op", bufs=5) as op,
            tc.tile_pool(name="psp", bufs=8, space="PSUM") as psp,
        ):
            # consts ride the ACT HWDGE ring so the SP ring's FIFO head is
            # the first window DMA (shaves the pipeline ramp)
            wmat_sb = constp.tile([128, BPC * 4], f32, tag="wmat")
            nc.scalar.dma_start(
                wmat_sb[:].rearrange("p (i q) -> p i q", q=4),
                wmat_d[:, :, :].transpose([1, 0, 2]),
            )
            offs_sb = constp.tile([1, BPC], i32, tag="offs")
            nc.scalar.dma_start(offs_sb[:], offs_d[:, :])
            smat_sb = constp.tile([128, 128], f32, tag="smat")
            nc.scalar.dma_start(smat_sb[:], smat_d[:, :])

            regs = [nc.alloc_register(mybir.EngineType.SP, f"dynoff_{k}")
                    for k in range(min(16, BPC))]
            svs = [nc.snap(r, donate=True, min_val=0, max_val=XLEN - 1)
                   for r in regs]
            nreg = len(regs)

            for i in range(BPC):
                k = i % nreg
                nc.sync.reg_load(regs[k], offs_sb[0:1, i: i + 1])
                wt = winp.tile([128, SPAN], f32, tag="wt")
                nc.sync.dma_start(
                    wt[:],
                    bass.AP(x_d, svs[k], [[WR * WPAD, 128], [1, SPAN]]),
                )
                # all operands are full-width CONTIGUOUS slices (junk
                # between rows is computed and trimmed on host): DVE runs
                # flat APs at full rate, strided 3D ones at half rate.
                # Shared horizontal lerp h over the whole span, then a
                # partition-local vertical lerp of h against h-shifted-by-
                # one-row: 4 passes total (2 ACT muls + 2 DVE madds).
                # ratio-form lerp, scale applied LAST by ACT (which runs
                # strided APs at full rate) into a COMPACT 512-wide output:
                #   g = wt + rx*wt[+1];  m = g + ry*g[+wpad];  y = S*m
                rx = wmat_sb[:, 4 * i + 0: 4 * i + 1]
                ry = wmat_sb[:, 4 * i + 1: 4 * i + 2]
                sc = wmat_sb[:, 4 * i + 2: 4 * i + 3]

                HL = SPAN - 1  # = WIDE: g rows 0..WR-1
                W3 = (WR - 1) * WPAD
                g = hp.tile([128, HL], f32, tag="g")
                m = mp.tile([128, W3 + N], f32, tag="m")
                oc = op.tile([128, WR * N], f32, tag="oc")
                ps = psp.tile([128, N], f32, tag="ps")

                nc.vector.scalar_tensor_tensor(g[:], wt[:, 1:HL + 1], rx,
                                               wt[:, 0:HL], MUL, ADD)
                # g row WR (= next partition's g row 0) via idle-PE partition
                # shift: ps[q, j] = g[q+1, j].  Global row N-1+1 has no next
                # partition -- that one output row is patched on host.
                nc.tensor.matmul(out=ps[:], lhsT=smat_sb[:, :],
                                 rhs=g[:, 0:N], start=True, stop=True)
                nc.vector.scalar_tensor_tensor(
                    m[:, 0:W3], g[:, WPAD:WR * WPAD], ry,
                    g[:, 0:W3], MUL, ADD)
                # smat column 127 is all-zero, so ps[127,:] = 0 and
                # partition 127 passes g through (host-patched row anyway)
                nc.vector.scalar_tensor_tensor(
                    m[:, W3:W3 + N], ps[:], ry,
                    g[:, W3:W3 + N], MUL, ADD)
                nc.scalar.mul(
                    oc[:, 0:(WR - 1) * N].rearrange("p (u j) -> p u j", j=N),
                    m[:, 0:W3].rearrange("p (u j) -> p u j", j=WPAD)[:, :, 0:N],
                    sc)
                nc.scalar.mul(oc[:, (WR - 1) * N:WR * N], m[:, W3:W3 + N], sc)

                nc.gpsimd.dma_start(
                    bass.AP(y_d, i * (N * N), [[WR * N, 128], [1, WR * N]]),
                    oc[:],
                )
    nc.compile()
    return nc


def host_prep(padded: np.ndarray, positions: np.ndarray, n_cores: int):
    """Shard + build metadata. padded: (B, npad, npad) f32, positions: (B, 2)."""
    B, npad, _ = padded.shape
    n = npad - 10
    bpc = B // n_cores

    px = positions[:, 0].astype(np.float32)
    py = positions[:, 1].astype(np.float32)
    fy = np.floor(py)
    fx = np.floor(px)
    ay = (5 + fy).astype(np.int64)
    ax = (5 + fx).astype(np.int64)
    wy = (py - fy).astype(np.float32)
    wx = (px - fx).astype(np.float32)

    m_lo = int(max(0, -min(ay.min(), ax.min())))
    m_hi = int(max(0, max(ay.max(), ax.max()) + n + 1 - npad))
    wpad = npad + m_lo + m_hi

    pp = np.zeros((B, wpad, wpad), dtype=np.float32)
    pp[:, m_lo:m_lo + npad, m_lo:m_lo + npad] = padded

    A = ay + m_lo
    Bc = ax + m_lo
    base = (np.arange(B, dtype=np.int64) % bpc) * (wpad * wpad)
    off = base + A * wpad + Bc

    wr = n // 128
    span = wr * wpad + 1
    # flat length incl. tail so the last image's strided span stays in bounds
    need = int(off.max()) + 127 * wr * wpad + span
    xlen = max(bpc * wpad * wpad, need)

    cfg = Cfg(bpc=bpc, n=n, wpad=wpad, xlen=xlen)

    smat = np.zeros((128, 128), dtype=np.float32)
    for m in range(127):
        smat[m + 1, m] = 1.0  # ps[m, j] = sum_k smat[k, m] g[k, j] = g[m+1, j]
    # column 127 stays zero: ps[127,:] = 0 (that row is host-patched)

    # host-side fixup for the last output row (needs input row A+n, which the
    # 4-row spans don't load)
    ar = np.arange(B)[:, None]
    ci = Bc[:, None] + np.arange(n + 1)[None, :]
    r0 = pp[ar, (A + n - 1)[:, None], ci]  # (B, n+1)
    r1 = pp[ar, (A + n)[:, None], ci]
    h0r = (1 - wx)[:, None] * r0[:, :n] + wx[:, None] * r0[:, 1:]
    h1r = (1 - wx)[:, None] * r1[:, :n] + wx[:, None] * r1[:, 1:]
    last_row = ((1 - wy)[:, None] * h0r + wy[:, None] * h1r).astype(np.float32)

    in_maps = []
    for cidx in range(n_cores):
        sl = slice(cidx * bpc, (cidx + 1) * bpc)
        flat = np.zeros((1, xlen), dtype=np.float32)
        flat[0, :bpc * wpad * wpad] = pp[sl].reshape(-1)
        offs = off[sl].astype(np.int32).reshape(1, bpc)
        wmat = np.empty((bpc, 128, 4), dtype=np.float32)
        wmat[:, :, 0] = (wx / (1 - wx))[sl][:, None]
        wmat[:, :, 1] = (wy / (1 - wy))[sl][:, None]
        wmat[:, :, 2] = ((1 - wx) * (1 - wy))[sl][:, None]
        wmat[:, :, 3] = 0.0
        in_maps.append({"x": flat, "offs": offs, "wmat": wmat, "smat": smat})
    return cfg, in_maps, last_row


N_CORES = 8
_nc_cache: dict = {}


def kernel(padded_obj: np.ndarray, positions: np.ndarray) -> np.ndarray:
    padded_obj = np.asarray(padded_obj)
    positions = np.asarray(positions)
    B, npad, _, C = padded_obj.shape
    cfg, in_maps, last_row = host_prep(
        padded_obj.reshape(B, npad, npad).astype(np.float32, copy=False),
        positions, N_CORES)

    nc = _nc_cache.get(cfg)
    if nc is None:
        nc = build_nc(cfg)
        _nc_cache[cfg] = nc

    res = run_bass_kernel_spmd(nc, in_maps, core_ids=list(range(N_CORES)))
    out = np.concatenate([r["y"][:, :, :cfg.n] for r in res.results], axis=0)
    out = np.ascontiguousarray(out)
    out[:, cfg.n - 1, :] = last_row
    return out.reshape(B, cfg.n, cfg.n, 1)



# revision 6
# speedup vs baseline: 1.0210x; 1.0210x over previous
"""Trainium2 Bass kernel for nn_ExtractPatchesPositionLayer.

Reference semantics: per image b, bilinear-translate the (522,522,1) padded
object by t = -positions[b] (tfa.translate: out(y,x) = img(y+py, x+px),
zero fill outside), then center-crop 5px -> (512,512,1).

Because the shift is constant per image, floor/frac of the offset give an
integer window start (A,B) into the (zero-margin-padded) image plus four
constant bilinear corner weights:

    out[r, j] = c00*W[r, j] + c01*W[r, j+1] + c10*W[r+1, j] + c11*W[r+1, j+1]
    W[r, c] = pp[A+r, B+c]

Layout trick: SBUF partition p holds FOUR consecutive padded-image rows
(A+4p .. A+4p+3, +1 elem) as ONE contiguous DRAM span (4*wpad+1 elements, a
single ~8.4 KB line-rate DMA descriptor per partition).  The shared
horizontal lerp h = (1-wx)*wt + wx*wt[+1] is computed once over the whole
span in RATIO form (one DVE fused madd: g = wt + rx*wt[+1], rx = wx/(1-wx);
all CONTIGUOUS free-dim APs -- DVE runs flat APs at ~2x the rate of strided
3D ones).  The vertical lerp is partition-local (m = g + ry*g[+wpad]) except
each partition's LAST row pair, whose g row 4 == next partition's g row 0:
the otherwise-idle PE recovers it with a shift-matrix matmul (zero last
column, so ps[127,:]=0 stays defined) that the DVE madd reads straight from
PSUM.  The combined scale S=(1-wx)(1-wy) is applied LAST by ACT, which runs
strided APs at full rate and therefore writes a COMPACT 512-wide output
tile: 4 consecutive y rows per partition = one contiguous 8 KB descriptor,
no write junk, no host trim.  The very last output row (needs input row
A+512, outside the spans) is patched on host -- O(B*N) work.  Ratio form is
numerically safe here: 1-wx, 1-wy in (0,1], and the big term dominates both
g and the output, keeping error at the output's ulp scale.

DMA routing (hard-won trace facts):
  * inputs: dynamic HWDGE on the SP ring (runtime reg offsets; descriptors
    spread over all 16 SDMA engines by dest SBUF partition).
  * outputs: SWDGE via gpsimd -- HWDGE sends every SBUF->HBM descriptor to
    SDMA engine 0 (1.4 ms serialized); SWDGE's CounterMachine spreads them.
    8+ KB descriptors avoid SWDGE's 8-byte stub-packet flood seen at 2 KB.
Sharding: batch 256 -> 32 images x 8 cores, embarrassingly parallel.
Measured: 1426 us (baseline banded-matmul PE kernel) -> 181 us best-state;
the 16 SDMA engines (~165 us busy each, work-conserving at ~100% mid-run)
are the binding resource (~400+ GB/s aggregate HBM traffic, past the
documented 358 GB/s per-core); DVE ~155 us.  Device power-state drift adds
up to ~12% run-to-run.
"""

from dataclasses import dataclass

import numpy as np

import concourse.bacc as bacc
import concourse.bass as bass
import concourse.mybir as mybir
import concourse.tile as tile
from concourse.bass_utils import run_bass_kernel_spmd


@dataclass(frozen=True)
class Cfg:
    bpc: int      # images per core
    n: int        # output height/width
    wpad: int     # padded input height/width (with zero margin)
    xlen: int     # flat padded-input length per core (incl. tail pad)

    @property
    def wrow(self):  # output rows per partition
        return self.n // 128

    @property
    def span(self):  # elements DMA'd per partition (WR rows + 1)
        return self.wrow * self.wpad + 1


def build_nc(cfg: Cfg) -> bass.Bass:
    BPC, N, WPAD = cfg.bpc, cfg.n, cfg.wpad
    WR = cfg.wrow
    SPAN = cfg.span
    WIDE = WR * WPAD  # full-width output row block per partition
    XLEN = cfg.xlen
    f32 = mybir.dt.float32
    f16 = mybir.dt.float16
    i32 = mybir.dt.int32
    MUL = mybir.AluOpType.mult
    ADD = mybir.AluOpType.add

    nc = bacc.Bacc("TRN2", target_bir_lowering=False, debug=False)
    x_d = nc.declare_dram_parameter("x", [1, XLEN], f16, isOutput=False)
    offs_d = nc.declare_dram_parameter("offs", [1, BPC], i32, isOutput=False)
    wmat_d = nc.declare_dram_parameter("wmat", [BPC, 128, 4], f16, isOutput=False)
    smat_d = nc.declare_dram_parameter("smat", [128, 128], f16, isOutput=False)
    y_d = nc.declare_dram_parameter("y", [BPC, N, N], f16, isOutput=True)

    with tile.TileContext(nc) as tc:
        with (
            tc.tile_pool(name="const", bufs=1) as constp,
            tc.tile_pool(name="win", bufs=8) as winp,
            tc.tile_pool(name="hp", bufs=5) as hp,
            tc.tile_pool(name="mp", bufs=4) as mp,
            tc.tile_pool(name="op", bufs=5) as op,
            tc.tile_pool(name="psp", bufs=8, space="PSUM") as psp,
        ):
            # consts ride the ACT HWDGE ring so the SP ring's FIFO head is
            # the first window DMA (shaves the pipeline ramp)
            wmat_sb = constp.tile([128, BPC * 4], f16, tag="wmat")
            nc.scalar.dma_start(
                wmat_sb[:].rearrange("p (i q) -> p i q", q=4),
                wmat_d[:, :, :].transpose([1, 0, 2]),
            )
            offs_sb = constp.tile([1, BPC], i32, tag="offs")
            nc.scalar.dma_start(offs_sb[:], offs_d[:, :])
            smat_sb = constp.tile([128, 128], f16, tag="smat")
            nc.scalar.dma_start(smat_sb[:], smat_d[:, :])

            regs = [nc.alloc_register(mybir.EngineType.SP, f"dynoff_{k}")
                    for k in range(min(16, BPC))]
            svs = [nc.snap(r, donate=True, min_val=0, max_val=XLEN - 1)
                   for r in regs]
            nreg = len(regs)

            for i in range(BPC):
                k = i % nreg
                nc.sync.reg_load(regs[k], offs_sb[0:1, i: i + 1])
                wt = winp.tile([128, SPAN], f16, tag="wt")
                nc.sync.dma_start(
                    wt[:],
                    bass.AP(x_d, svs[k], [[WR * WPAD, 128], [1, SPAN]]),
                )
                # all operands are full-width CONTIGUOUS slices (junk
                # between rows is computed and trimmed on host): DVE runs
                # flat APs at full rate, strided 3D ones at half rate.
                # Shared horizontal lerp h over the whole span, then a
                # partition-local vertical lerp of h against h-shifted-by-
                # one-row: 4 passes total (2 ACT muls + 2 DVE madds).
                # ratio-form lerp, scale applied LAST by ACT (which runs
                # strided APs at full rate) into a COMPACT 512-wide output:
                #   g = wt + rx*wt[+1];  m = g + ry*g[+wpad];  y = S*m
                rx = wmat_sb[:, 4 * i + 0: 4 * i + 1]
                ry = wmat_sb[:, 4 * i + 1: 4 * i + 2]

                HL = SPAN - 1  # = WIDE: g rows 0..WR-1
                W3 = (WR - 1) * WPAD
                g = hp.tile([128, HL], f16, tag="g")
                m = mp.tile([128, W3 + N], f16, tag="m")
                oc = op.tile([128, WR * N], f16, tag="oc")
                ps = psp.tile([128, N], f32, tag="ps")

                nc.vector.scalar_tensor_tensor(g[:], wt[:, 1:HL + 1], rx,
                                               wt[:, 0:HL], MUL, ADD)
                # g row WR (= next partition's g row 0) via idle-PE partition
                # shift: ps[q, j] = g[q+1, j].  Global row N-1+1 has no next
                # partition -- that one output row is patched on host.
                nc.tensor.matmul(out=ps[:], lhsT=smat_sb[:, :],
                                 rhs=g[:, 0:N], start=True, stop=True)
                nc.vector.scalar_tensor_tensor(
                    m[:, 0:W3], g[:, WPAD:WR * WPAD], ry,
                    g[:, 0:W3], MUL, ADD)
                # smat column 127 is all-zero, so ps[127,:] = 0 and
                # partition 127 passes g through (host-patched row anyway)
                nc.vector.scalar_tensor_tensor(
                    m[:, W3:W3 + N], ps[:], ry,
                    g[:, W3:W3 + N], MUL, ADD)
                # host pre-scaled by S: ACT is a pure compaction copy now
                # (scale literal 1.0 -- ACT requires an FP32 scale AP, and
                # the f16 wmat slice is rejected by the BIR verifier)
                nc.scalar.mul(
                    oc[:, 0:(WR - 1) * N].rearrange("p (u j) -> p u j", j=N),
                    m[:, 0:W3].rearrange("p (u j) -> p u j", j=WPAD)[:, :, 0:N],
                    1.0)
                nc.scalar.mul(oc[:, (WR - 1) * N:WR * N], m[:, W3:W3 + N], 1.0)

                nc.gpsimd.dma_start(
                    bass.AP(y_d, i * (N * N), [[WR * N, 128], [1, WR * N]]),
                    oc[:],
                )
    nc.compile()
    return nc


def host_prep(padded: np.ndarray, positions: np.ndarray, n_cores: int):
    """Shard + build metadata. padded: (B, npad, npad) f32, positions: (B, 2)."""
    B, npad, _ = padded.shape
    n = npad - 10
    bpc = B // n_cores

    px = positions[:, 0].astype(np.float32)
    py = positions[:, 1].astype(np.float32)
    fy = np.floor(py)
    fx = np.floor(px)
    ay = (5 + fy).astype(np.int64)
    ax = (5 + fx).astype(np.int64)
    wy = (py - fy).astype(np.float32)
    wx = (px - fx).astype(np.float32)

    m_lo = int(max(0, -min(ay.min(), ax.min())))
    m_hi = int(max(0, max(ay.max(), ax.max()) + n + 1 - npad))
    wpad = npad + m_lo + m_hi

    pp = np.zeros((B, wpad, wpad), dtype=np.float32)
    pp[:, m_lo:m_lo + npad, m_lo:m_lo + npad] = padded

    A = ay + m_lo
    Bc = ax + m_lo
    base = (np.arange(B, dtype=np.int64) % bpc) * (wpad * wpad)
    off = base + A * wpad + Bc

    wr = n // 128
    span = wr * wpad + 1
    # flat length incl. tail so the last image's strided span stays in bounds
    need = int(off.max()) + 127 * wr * wpad + span
    xlen = max(bpc * wpad * wpad, need)

    cfg = Cfg(bpc=bpc, n=n, wpad=wpad, xlen=xlen)

    smat = np.zeros((128, 128), dtype=np.float32)
    for m in range(127):
        smat[m + 1, m] = 1.0  # ps[m, j] = sum_k smat[k, m] g[k, j] = g[m+1, j]
    # column 127 stays zero: ps[127,:] = 0 (that row is host-patched)

    # host-side fixup for the last output row (needs input row A+n, which the
    # 4-row spans don't load)
    ar = np.arange(B)[:, None]
    ci = Bc[:, None] + np.arange(n + 1)[None, :]
    r0 = pp[ar, (A + n - 1)[:, None], ci]  # (B, n+1)
    r1 = pp[ar, (A + n)[:, None], ci]
    h0r = (1 - wx)[:, None] * r0[:, :n] + wx[:, None] * r0[:, 1:]
    h1r = (1 - wx)[:, None] * r1[:, :n] + wx[:, None] * r1[:, 1:]
    last_row = ((1 - wy)[:, None] * h0r + wy[:, None] * h1r).astype(np.float32)

    # fp16 I/O: pre-scale each image by S=(1-wx)(1-wy) on the host so the
    # ratio-form intermediates are bounded (g <= max|x|*(1-wy), m <= max|x|)
    # -- no fp16 overflow regardless of wx,wy -- and the ACT pass becomes a
    # pure compaction (sc=1).  fp16 quantization costs ~1e-3 rel err, far
    # under the 2e-2 gate, and halves HBM traffic (the binding resource).
    S = ((1 - wx) * (1 - wy)).astype(np.float32)

    in_maps = []
    for cidx in range(n_cores):
        sl = slice(cidx * bpc, (cidx + 1) * bpc)
        flat = np.zeros((1, xlen), dtype=np.float16)
        flat[0, :bpc * wpad * wpad] = (
            pp[sl] * S[sl][:, None, None]).astype(np.float16).reshape(-1)
        offs = off[sl].astype(np.int32).reshape(1, bpc)
        wmat = np.empty((bpc, 128, 4), dtype=np.float16)
        wmat[:, :, 0] = (wx / (1 - wx))[sl].astype(np.float16)[:, None]
        wmat[:, :, 1] = (wy / (1 - wy))[sl].astype(np.float16)[:, None]
        wmat[:, :, 2] = 1.0
        wmat[:, :, 3] = 0.0
        in_maps.append({"x": flat, "offs": offs, "wmat": wmat,
                        "smat": smat.astype(np.float16)})
    return cfg, in_maps, last_row


N_CORES = 8
_nc_cache: dict = {}


def kernel(padded_obj: np.ndarray, positions: np.ndarray) -> np.ndarray:
    padded_obj = np.asarray(padded_obj)
    positions = np.asarray(positions)
    B, npad, _, C = padded_obj.shape
    cfg, in_maps, last_row = host_prep(
        padded_obj.reshape(B, npad, npad).astype(np.float32, copy=False),
        positions, N_CORES)

    nc = _nc_cache.get(cfg)
    if nc is None:
        nc = build_nc(cfg)
        _nc_cache[cfg] = nc

    res = run_bass_kernel_spmd(nc, in_maps, core_ids=list(range(N_CORES)))
    out = np.concatenate(
        [r["y"][:, :, :cfg.n] for r in res.results], axis=0).astype(np.float32)
    out[:, cfg.n - 1, :] = last_row
    return out.reshape(B, cfg.n, cfg.n, 1)



# revision 7
# speedup vs baseline: 1.5165x; 1.4854x over previous
"""Trainium2 Bass kernel for nn_ExtractPatchesPositionLayer.

Reference semantics: per image b, bilinear-translate the (522,522,1) padded
object by t = -positions[b] (tfa.translate: out(y,x) = img(y+py, x+px),
zero fill outside), then center-crop 5px -> (512,512,1).

Because the shift is constant per image, floor/frac of the offset give an
integer window start (A,B) into the (zero-margin-padded) image plus four
constant bilinear corner weights:

    out[r, j] = c00*W[r, j] + c01*W[r, j+1] + c10*W[r+1, j] + c11*W[r+1, j+1]
    W[r, c] = pp[A+r, B+c]

fp16 I/O (the 2e-2 rel-err gate dwarfs fp16's ~3e-4): halves HBM traffic,
the prior binding resource.  Host pre-scales each image by S=(1-wx)(1-wy),
so the ratio-form lerp (rx=wx/(1-wx), ry=wy/(1-wy)) needs no final scale
and every intermediate is bounded by max|x| -- no fp16 overflow even at
rx~6e3 (S*(1+rx)(1+ry)=1).  Subnormal-f16 behaviour verified on HW
(rel err 4e-4 measured with the extreme-S seed-0 image).

Engine split (from the fp16 trace: DVE stt was 153us/core = the bottleneck;
stt = ScalarTensorTensor has NO fast DVE modes, 1 elem/cycle @0.96GHz):
  ACT  t0c = rx*wt[+1]   strided 3D read of the 4-row span, COMPACT write
       (fuses the old standalone compaction pass into the h-lerp mul; ACT
       runs strided APs at full rate, 1/cycle @1.2GHz, scale AP must be f32)
  DVE  gc  = wt[3D] + t0c     TensorTensor add: 2x_1p mode (fp16 packed)
  PE   ps  = shift @ gc[:,0:N]    boundary row (partition p+1's row 0)
  ACT  t1c[3N:4N] = ry*ps         PSUM read, f16 out
  DVE  t1c[0:3N]  = ry*gc[N:4N]   TensorScalarPtr: 4x_2p mode (all-SBUF fp16)
  DVE  oc  = gc + t1c             TensorTensor add 2x_1p, all flat
Predicted busy/core: DVE ~2514 cyc/img -> ~98us, ACT ~2.5us/img -> ~80us,
DMA ~95us (22.5 GB/s per SDMA engine line rate x16).

DMA routing (hard-won trace facts):
  * inputs: dynamic HWDGE on the SP ring (runtime reg offsets; descriptors
    spread over all 16 SDMA engines by dest SBUF partition).
  * outputs: SWDGE via gpsimd -- HWDGE sends every SBUF->HBM descriptor to
    SDMA engine 0 (serialized); SWDGE's CounterMachine spreads them.
Sharding: batch 256 -> 32 images x 8 cores, embarrassingly parallel.
History: 1426us (banded-matmul PE) -> 181us (f32 stt kernel) -> 178us
(fp16 stt; DVE-bound) -> this restructure.
"""

from dataclasses import dataclass

import numpy as np

import concourse.bacc as bacc
import concourse.bass as bass
import concourse.mybir as mybir
import concourse.tile as tile
from concourse.bass_utils import run_bass_kernel_spmd


@dataclass(frozen=True)
class Cfg:
    bpc: int      # images per core
    n: int        # output height/width
    wpad: int     # padded input height/width (with zero margin)
    xlen: int     # flat padded-input length per core (incl. tail pad)

    @property
    def wrow(self):  # output rows per partition
        return self.n // 128

    @property
    def span(self):  # elements DMA'd per partition (WR rows + 1)
        return self.wrow * self.wpad + 1


def build_nc(cfg: Cfg) -> bass.Bass:
    BPC, N, WPAD = cfg.bpc, cfg.n, cfg.wpad
    WR = cfg.wrow
    SPAN = cfg.span
    XLEN = cfg.xlen
    OC = WR * N  # compact elems per partition (4 rows x 512)
    f32 = mybir.dt.float32
    f16 = mybir.dt.float16
    i32 = mybir.dt.int32

    nc = bacc.Bacc("TRN2", target_bir_lowering=False, debug=False)
    x_d = nc.declare_dram_parameter("x", [1, XLEN], f16, isOutput=False)
    offs_d = nc.declare_dram_parameter("offs", [1, BPC], i32, isOutput=False)
    wmat_d = nc.declare_dram_parameter("wmat", [BPC, 128, 4], f32, isOutput=False)
    smat_d = nc.declare_dram_parameter("smat", [128, 128], f16, isOutput=False)
    y_d = nc.declare_dram_parameter("y", [BPC, N, N], f16, isOutput=True)

    with tile.TileContext(nc) as tc:
        with (
            tc.tile_pool(name="const", bufs=1) as constp,
            tc.tile_pool(name="win", bufs=8) as winp,
            tc.tile_pool(name="t0p", bufs=4) as t0p,
            tc.tile_pool(name="gp", bufs=4) as gp,
            tc.tile_pool(name="tp", bufs=4) as tp,
            tc.tile_pool(name="op", bufs=5) as op,
            tc.tile_pool(name="psp", bufs=8, space="PSUM") as psp,
        ):
            # consts ride the ACT HWDGE ring so the SP ring's FIFO head is
            # the first window DMA (shaves the pipeline ramp)
            wmat_sb = constp.tile([128, BPC * 4], f32, tag="wmat")
            nc.scalar.dma_start(
                wmat_sb[:].rearrange("p (i q) -> p i q", q=4),
                wmat_d[:, :, :].transpose([1, 0, 2]),
            )
            offs_sb = constp.tile([1, BPC], i32, tag="offs")
            nc.scalar.dma_start(offs_sb[:], offs_d[:, :])
            smat_sb = constp.tile([128, 128], f16, tag="smat")
            nc.scalar.dma_start(smat_sb[:], smat_d[:, :])

            regs = [nc.alloc_register(mybir.EngineType.SP, f"dynoff_{k}")
                    for k in range(min(16, BPC))]
            svs = [nc.snap(r, donate=True, min_val=0, max_val=XLEN - 1)
                   for r in regs]
            nreg = len(regs)

            for i in range(BPC):
                k = i % nreg
                nc.sync.reg_load(regs[k], offs_sb[0:1, i: i + 1])
                wt = winp.tile([128, SPAN], f16, tag="wt")
                nc.sync.dma_start(
                    wt[:],
                    bass.AP(x_d, svs[k], [[WR * WPAD, 128], [1, SPAN]]),
                )
                rx = wmat_sb[:, 4 * i + 0: 4 * i + 1]  # f32 scalar APs
                ry = wmat_sb[:, 4 * i + 1: 4 * i + 2]

                t0c = t0p.tile([128, OC], f16, tag="t0c")
                gc = gp.tile([128, OC], f16, tag="gc")
                t1c = tp.tile([128, OC], f16, tag="t1c")
                oc = op.tile([128, OC], f16, tag="oc")
                ps = psp.tile([128, N], f32, tag="ps")

                # 3D views of the span: row r=0..WR-1, cols 0..N-1 (+shift)
                wt0 = wt[:, 0:WR * WPAD].rearrange(
                    "p (r c) -> p r c", c=WPAD)[:, :, 0:N]
                wt1 = wt[:, 1:WR * WPAD + 1].rearrange(
                    "p (r c) -> p r c", c=WPAD)[:, :, 0:N]
                t0c3 = t0c[:].rearrange("p (r c) -> p r c", c=N)
                gc3 = gc[:].rearrange("p (r c) -> p r c", c=N)

                # ACT: shifted h-lerp term, strided->compact (fused old
                # compaction); scale AP must be FP32
                nc.scalar.mul(t0c3, wt1, rx)
                # DVE 2x: gc = wt + t0c  (h-lerp done, compact layout)
                nc.vector.tensor_add(gc3, wt0, t0c3)
                # PE: boundary row ps[p,:] = gc[p+1, 0:N] (image row 4p+4);
                # smat col 127 all-zero -> ps[127,:]=0 (host-patched row)
                nc.tensor.matmul(out=ps[:], lhsT=smat_sb[:, :],
                                 rhs=gc[:, 0:N], start=True, stop=True)
                # ACT: boundary v-lerp term from PSUM
                nc.scalar.mul(t1c[:, (WR - 1) * N:OC], ps[:], ry)
                # DVE 4x: interior v-lerp terms, all flat SBUF
                nc.vector.tensor_scalar_mul(
                    t1c[:, 0:(WR - 1) * N], gc[:, N:OC], ry)
                # DVE 2x: final add, all flat
                nc.vector.tensor_add(oc[:], gc[:], t1c[:])

                nc.gpsimd.dma_start(
                    bass.AP(y_d, i * (N * N), [[WR * N, 128], [1, WR * N]]),
                    oc[:],
                )
    nc.compile()
    return nc


def host_prep(padded: np.ndarray, positions: np.ndarray, n_cores: int):
    """Shard + build metadata. padded: (B, npad, npad) f32, positions: (B, 2)."""
    B, npad, _ = padded.shape
    n = npad - 10
    bpc = B // n_cores

    px = positions[:, 0].astype(np.float32)
    py = positions[:, 1].astype(np.float32)
    fy = np.floor(py)
    fx = np.floor(px)
    ay = (5 + fy).astype(np.int64)
    ax = (5 + fx).astype(np.int64)
    wy = (py - fy).astype(np.float32)
    wx = (px - fx).astype(np.float32)

    m_lo = int(max(0, -min(ay.min(), ax.min())))
    m_hi = int(max(0, max(ay.max(), ax.max()) + n + 1 - npad))
    wpad = npad + m_lo + m_hi

    pp = np.zeros((B, wpad, wpad), dtype=np.float32)
    pp[:, m_lo:m_lo + npad, m_lo:m_lo + npad] = padded

    A = ay + m_lo
    Bc = ax + m_lo
    base = (np.arange(B, dtype=np.int64) % bpc) * (wpad * wpad)
    off = base + A * wpad + Bc

    wr = n // 128
    span = wr * wpad + 1
    # flat length incl. tail so the last image's strided span stays in bounds
    need = int(off.max()) + 127 * wr * wpad + span
    xlen = max(bpc * wpad * wpad, need)

    cfg = Cfg(bpc=bpc, n=n, wpad=wpad, xlen=xlen)

    smat = np.zeros((128, 128), dtype=np.float32)
    for m in range(127):
        smat[m + 1, m] = 1.0  # ps[m, j] = sum_k smat[k, m] g[k, j] = g[m+1, j]
    # column 127 stays zero: ps[127,:] = 0 (that row is host-patched)

    # host-side fixup for the last output row (needs input row A+n, which the
    # 4-row spans don't load)
    ar = np.arange(B)[:, None]
    ci = Bc[:, None] + np.arange(n + 1)[None, :]
    r0 = pp[ar, (A + n - 1)[:, None], ci]  # (B, n+1)
    r1 = pp[ar, (A + n)[:, None], ci]
    h0r = (1 - wx)[:, None] * r0[:, :n] + wx[:, None] * r0[:, 1:]
    h1r = (1 - wx)[:, None] * r1[:, :n] + wx[:, None] * r1[:, 1:]
    last_row = ((1 - wy)[:, None] * h0r + wy[:, None] * h1r).astype(np.float32)

    # fp16 I/O: pre-scale each image by S=(1-wx)(1-wy) on the host so the
    # ratio-form intermediates are bounded (gc <= max|x|*(1-wy), oc <=
    # max|x|) -- no fp16 overflow regardless of wx,wy -- and no final scale
    # is needed.  fp16 quantization costs ~3e-4 rel err vs the 2e-2 gate.
    S = ((1 - wx) * (1 - wy)).astype(np.float32)

    in_maps = []
    for cidx in range(n_cores):
        sl = slice(cidx * bpc, (cidx + 1) * bpc)
        flat = np.zeros((1, xlen), dtype=np.float16)
        flat[0, :bpc * wpad * wpad] = (
            pp[sl] * S[sl][:, None, None]).astype(np.float16).reshape(-1)
        offs = off[sl].astype(np.int32).reshape(1, bpc)
        wmat = np.empty((bpc, 128, 4), dtype=np.float32)
        wmat[:, :, 0] = (wx / (1 - wx))[sl][:, None]
        wmat[:, :, 1] = (wy / (1 - wy))[sl][:, None]
        wmat[:, :, 2] = 1.0
        wmat[:, :, 3] = 0.0
        in_maps.append({"x": flat, "offs": offs, "wmat": wmat,
                        "smat": smat.astype(np.float16)})
    return cfg, in_maps, last_row


N_CORES = 8
_nc_cache: dict = {}


def kernel(padded_obj: np.ndarray, positions: np.ndarray) -> np.ndarray:
    padded_obj = np.asarray(padded_obj)
    positions = np.asarray(positions)
    B, npad, _, C = padded_obj.shape
    cfg, in_maps, last_row = host_prep(
        padded_obj.reshape(B, npad, npad).astype(np.float32, copy=False),
        positions, N_CORES)

    nc = _nc_cache.get(cfg)
    if nc is None:
        nc = build_nc(cfg)
        _nc_cache[cfg] = nc

    res = run_bass_kernel_spmd(nc, in_maps, core_ids=list(range(N_CORES)))
    out = np.concatenate(
        [r["y"][:, :, :cfg.n] for r in res.results], axis=0).astype(np.float32)
    out[:, cfg.n - 1, :] = last_row
    return out.reshape(B, cfg.n, cfg.n, 1)


# revision 8
# speedup vs baseline: 1.5280x; 1.0076x over previous
"""Trainium2 Bass kernel for nn_ExtractPatchesPositionLayer.

Reference semantics: per image b, bilinear-translate the (522,522,1) padded
object by t = -positions[b] (tfa.translate: out(y,x) = img(y+py, x+px),
zero fill outside), then center-crop 5px -> (512,512,1).

Because the shift is constant per image, floor/frac of the offset give an
integer window start (A,B) into the (zero-margin-padded) image plus four
constant bilinear corner weights:

    out[r, j] = c00*W[r, j] + c01*W[r, j+1] + c10*W[r+1, j] + c11*W[r+1, j+1]
    W[r, c] = pp[A+r, B+c]

fp16 I/O (the 2e-2 rel-err gate dwarfs fp16's ~3e-4): halves HBM traffic,
the prior binding resource.  Host pre-scales each image by S=(1-wx)(1-wy),
so the ratio-form lerp (rx=wx/(1-wx), ry=wy/(1-wy)) needs no final scale
and every intermediate is bounded by max|x| -- no fp16 overflow even at
rx~6e3 (S*(1+rx)(1+ry)=1).  Subnormal-f16 behaviour verified on HW
(rel err 4e-4 measured with the extreme-S seed-0 image).

Engine split (from the fp16 trace: DVE stt was 153us/core = the bottleneck;
stt = ScalarTensorTensor has NO fast DVE modes, 1 elem/cycle @0.96GHz):
  ACT  t0c = rx*wt[+1]   strided 3D read of the 4-row span, COMPACT write
       (fuses the old standalone compaction pass into the h-lerp mul; ACT
       runs strided APs at full rate, 1/cycle @1.2GHz, scale AP must be f32)
  DVE  gc  = wt[3D] + t0c     TensorTensor add: 2x_1p mode (fp16 packed)
  PE   ps  = shift @ gc[:,0:N]    boundary row (partition p+1's row 0)
  ACT  t1c[3N:4N] = ry*ps         PSUM read, f16 out
  DVE  t1c[0:3N]  = ry*gc[N:4N]   TensorScalarPtr: 4x_2p mode (all-SBUF fp16)
  DVE  oc  = gc + t1c             TensorTensor add 2x_1p, all flat
Predicted busy/core: DVE ~2514 cyc/img -> ~98us, ACT ~2.5us/img -> ~80us,
DMA ~95us (22.5 GB/s per SDMA engine line rate x16).

DMA routing (hard-won trace facts):
  * inputs: dynamic HWDGE on the SP ring (runtime reg offsets; descriptors
    spread over all 16 SDMA engines by dest SBUF partition).
  * outputs: SWDGE via gpsimd -- HWDGE sends every SBUF->HBM descriptor to
    SDMA engine 0 (serialized); SWDGE's CounterMachine spreads them.
Sharding: batch 256 -> 32 images x 8 cores, embarrassingly parallel.
History: 1426us (banded-matmul PE) -> 181us (f32 stt kernel) -> 178us
(fp16 stt; DVE-bound) -> this restructure.
"""

from dataclasses import dataclass

import numpy as np

import concourse.bacc as bacc
import concourse.bass as bass
import concourse.mybir as mybir
import concourse.tile as tile
from concourse.bass_utils import run_bass_kernel_spmd


@dataclass(frozen=True)
class Cfg:
    bpc: int      # images per core
    n: int        # output height/width
    wpad: int     # padded input height/width (with zero margin)
    xlen: int     # flat padded-input length per core (incl. tail pad)

    @property
    def wrow(self):  # output rows per partition
        return self.n // 128

    @property
    def span(self):  # elements DMA'd per partition (WR rows + 1)
        return self.wrow * self.wpad + 1


def build_nc(cfg: Cfg) -> bass.Bass:
    BPC, N, WPAD = cfg.bpc, cfg.n, cfg.wpad
    WR = cfg.wrow
    SPAN = cfg.span
    XLEN = cfg.xlen
    OC = WR * N  # compact elems per partition (4 rows x 512)
    f32 = mybir.dt.float32
    f16 = mybir.dt.float16
    i32 = mybir.dt.int32

    nc = bacc.Bacc("TRN2", target_bir_lowering=False, debug=False)
    x_d = nc.declare_dram_parameter("x", [1, XLEN], f16, isOutput=False)
    offs_d = nc.declare_dram_parameter("offs", [1, BPC], i32, isOutput=False)
    wmat_d = nc.declare_dram_parameter("wmat", [BPC, 128, 4], f32, isOutput=False)
    smat_d = nc.declare_dram_parameter("smat", [128, 128], f16, isOutput=False)
    y_d = nc.declare_dram_parameter("y", [BPC, N, N], f16, isOutput=True)

    with tile.TileContext(nc) as tc:
        with (
            tc.tile_pool(name="const", bufs=1) as constp,
            tc.tile_pool(name="win", bufs=8) as winp,
            tc.tile_pool(name="t0p", bufs=4) as t0p,
            tc.tile_pool(name="gp", bufs=4) as gp,
            tc.tile_pool(name="tp", bufs=4) as tp,
            tc.tile_pool(name="op", bufs=5) as op,
            tc.tile_pool(name="psp", bufs=8, space="PSUM") as psp,
        ):
            # consts ride the ACT HWDGE ring so the SP ring's FIFO head is
            # the first window DMA (shaves the pipeline ramp)
            wmat_sb = constp.tile([128, BPC * 4], f32, tag="wmat")
            nc.scalar.dma_start(
                wmat_sb[:].rearrange("p (i q) -> p i q", q=4),
                wmat_d[:, :, :].transpose([1, 0, 2]),
            )
            offs_sb = constp.tile([1, BPC], i32, tag="offs")
            nc.scalar.dma_start(offs_sb[:], offs_d[:, :])
            smat_sb = constp.tile([128, 128], f16, tag="smat")
            nc.scalar.dma_start(smat_sb[:], smat_d[:, :])

            regs = [nc.alloc_register(mybir.EngineType.SP, f"dynoff_{k}")
                    for k in range(min(16, BPC))]
            svs = [nc.snap(r, donate=True, min_val=0, max_val=XLEN - 1)
                   for r in regs]
            nreg = len(regs)

            # Software pipeline, skewed 2 stages: every engine op only
            # consumes tiles produced in an EARLIER iteration, so no engine
            # ever stalls on an intra-iteration cross-engine round trip
            # (the unskewed version lost ~28us/core to exactly that).
            #   iter i:  SP  dma wt[i]
            #            ACT t1r3[i-2]; t0c[i]
            #            DVE gc[i-1]; ts[i-1]; oc[i-2]
            #            PE  mm[i-1]
            #            GPS swdge[i-2]
            st = {}  # image idx -> dict of live tiles / scalars

            def stage_a(i):  # DMA + ACT h-lerp mul (strided->compact)
                k = i % nreg
                nc.sync.reg_load(regs[k], offs_sb[0:1, i: i + 1])
                wt = winp.tile([128, SPAN], f16, tag="wt")
                nc.sync.dma_start(
                    wt[:],
                    bass.AP(x_d, svs[k], [[WR * WPAD, 128], [1, SPAN]]),
                )
                t0c = t0p.tile([128, OC], f16, tag="t0c")
                wt1 = wt[:, 1:WR * WPAD + 1].rearrange(
                    "p (r c) -> p r c", c=WPAD)[:, :, 0:N]
                # scale AP must be FP32 for ACT
                nc.scalar.mul(t0c[:].rearrange("p (r c) -> p r c", c=N),
                              wt1, wmat_sb[:, 4 * i: 4 * i + 1])
                st[i] = {"wt": wt, "t0c": t0c,
                         "ry": wmat_sb[:, 4 * i + 1: 4 * i + 2]}

            def stage_b(i):  # DVE h-lerp add + v-lerp mul, PE boundary row
                s = st[i]
                wt0 = s["wt"][:, 0:WR * WPAD].rearrange(
                    "p (r c) -> p r c", c=WPAD)[:, :, 0:N]
                gc = gp.tile([128, OC], f16, tag="gc")
                t1c = tp.tile([128, OC], f16, tag="t1c")
                ps = psp.tile([128, N], f32, tag="ps")
                # DVE 2x: gc = wt + t0c  (h-lerp done, compact layout)
                nc.vector.tensor_add(
                    gc[:].rearrange("p (r c) -> p r c", c=N), wt0,
                    s["t0c"][:].rearrange("p (r c) -> p r c", c=N))
                # PE: boundary row ps[p,:] = gc[p+1, 0:N] (image row 4p+4);
                # smat col 127 all-zero -> ps[127,:]=0 (host-patched row)
                nc.tensor.matmul(out=ps[:], lhsT=smat_sb[:, :],
                                 rhs=gc[:, 0:N], start=True, stop=True)
                # DVE 4x: interior v-lerp terms, all flat SBUF
                nc.vector.tensor_scalar_mul(
                    t1c[:, 0:(WR - 1) * N], gc[:, N:OC], s["ry"])
                s.update(gc=gc, t1c=t1c, ps=ps)

            def stage_c_act(i):  # ACT: boundary v-lerp term from PSUM
                s = st[i]
                nc.scalar.mul(s["t1c"][:, (WR - 1) * N:OC], s["ps"][:],
                              s["ry"])

            def stage_c_rest(i):  # DVE final add + SWDGE out
                s = st.pop(i)
                oc = op.tile([128, OC], f16, tag="oc")
                # DVE 2x: final add, all flat
                nc.vector.tensor_add(oc[:], s["gc"][:], s["t1c"][:])
                nc.gpsimd.dma_start(
                    bass.AP(y_d, i * (N * N), [[WR * N, 128], [1, WR * N]]),
                    oc[:],
                )

            for i in range(BPC + 2):
                if i - 2 >= 0:
                    stage_c_act(i - 2)
                if i < BPC:
                    stage_a(i)
                if 0 <= i - 1 < BPC:
                    stage_b(i - 1)
                if i - 2 >= 0:
                    stage_c_rest(i - 2)
    nc.compile()
    return nc


def host_prep(padded: np.ndarray, positions: np.ndarray, n_cores: int):
    """Shard + build metadata. padded: (B, npad, npad) f32, positions: (B, 2)."""
    B, npad, _ = padded.shape
    n = npad - 10
    bpc = B // n_cores

    px = positions[:, 0].astype(np.float32)
    py = positions[:, 1].astype(np.float32)
    fy = np.floor(py)
    fx = np.floor(px)
    ay = (5 + fy).astype(np.int64)
    ax = (5 + fx).astype(np.int64)
    wy = (py - fy).astype(np.float32)
    wx = (px - fx).astype(np.float32)

    m_lo = int(max(0, -min(ay.min(), ax.min())))
    m_hi = int(max(0, max(ay.max(), ax.max()) + n + 1 - npad))
    wpad = npad + m_lo + m_hi

    pp = np.zeros((B, wpad, wpad), dtype=np.float32)
    pp[:, m_lo:m_lo + npad, m_lo:m_lo + npad] = padded

    A = ay + m_lo
    Bc = ax + m_lo
    base = (np.arange(B, dtype=np.int64) % bpc) * (wpad * wpad)
    off = base + A * wpad + Bc

    wr = n // 128
    span = wr * wpad + 1
    # flat length incl. tail so the last image's strided span stays in bounds
    need = int(off.max()) + 127 * wr * wpad + span
    xlen = max(bpc * wpad * wpad, need)

    cfg = Cfg(bpc=bpc, n=n, wpad=wpad, xlen=xlen)

    smat = np.zeros((128, 128), dtype=np.float32)
    for m in range(127):
        smat[m + 1, m] = 1.0  # ps[m, j] = sum_k smat[k, m] g[k, j] = g[m+1, j]
    # column 127 stays zero: ps[127,:] = 0 (that row is host-patched)

    # host-side fixup for the last output row (needs input row A+n, which the
    # 4-row spans don't load)
    ar = np.arange(B)[:, None]
    ci = Bc[:, None] + np.arange(n + 1)[None, :]
    r0 = pp[ar, (A + n - 1)[:, None], ci]  # (B, n+1)
    r1 = pp[ar, (A + n)[:, None], ci]
    h0r = (1 - wx)[:, None] * r0[:, :n] + wx[:, None] * r0[:, 1:]
    h1r = (1 - wx)[:, None] * r1[:, :n] + wx[:, None] * r1[:, 1:]
    last_row = ((1 - wy)[:, None] * h0r + wy[:, None] * h1r).astype(np.float32)

    # fp16 I/O: pre-scale each image by S=(1-wx)(1-wy) on the host so the
    # ratio-form intermediates are bounded (gc <= max|x|*(1-wy), oc <=
    # max|x|) -- no fp16 overflow regardless of wx,wy -- and no final scale
    # is needed.  fp16 quantization costs ~3e-4 rel err vs the 2e-2 gate.
    S = ((1 - wx) * (1 - wy)).astype(np.float32)

    in_maps = []
    for cidx in range(n_cores):
        sl = slice(cidx * bpc, (cidx + 1) * bpc)
        flat = np.zeros((1, xlen), dtype=np.float16)
        flat[0, :bpc * wpad * wpad] = (
            pp[sl] * S[sl][:, None, None]).astype(np.float16).reshape(-1)
        offs = off[sl].astype(np.int32).reshape(1, bpc)
        wmat = np.empty((bpc, 128, 4), dtype=np.float32)
        wmat[:, :, 0] = (wx / (1 - wx))[sl][:, None]
        wmat[:, :, 1] = (wy / (1 - wy))[sl][:, None]
        wmat[:, :, 2] = 1.0
        wmat[:, :, 3] = 0.0
        in_maps.append({"x": flat, "offs": offs, "wmat": wmat,
                        "smat": smat.astype(np.float16)})
    return cfg, in_maps, last_row


N_CORES = 8
_nc_cache: dict = {}


def kernel(padded_obj: np.ndarray, positions: np.ndarray) -> np.ndarray:
    padded_obj = np.asarray(padded_obj)
    positions = np.asarray(positions)
    B, npad, _, C = padded_obj.shape
    cfg, in_maps, last_row = host_prep(
        padded_obj.reshape(B, npad, npad).astype(np.float32, copy=False),
        positions, N_CORES)

    nc = _nc_cache.get(cfg)
    if nc is None:
        nc = build_nc(cfg)
        _nc_cache[cfg] = nc

    res = run_bass_kernel_spmd(nc, in_maps, core_ids=list(range(N_CORES)))
    out = np.concatenate(
        [r["y"][:, :, :cfg.n] for r in res.results], axis=0).astype(np.float32)
    out[:, cfg.n - 1, :] = last_row
    return out.reshape(B, cfg.n, cfg.n, 1)
